# revision 1
# baseline (speedup 1.0000x reference)
"""Multi-head attention (B=4, N=2048, D=768, H=12) on 8 trn2 NeuronCores.

Sharding: core c -> (batch b = c//2, head-half g = c%2).  Each core computes
the qkv projection for its 6 heads, attention, and a partial output
projection (over its 384 feature columns).  The host sums the two partials
per batch and adds the proj bias.  No collectives.

Device design (per core):
 - x is transposed on host to xT [768, 2048] so the contraction dim (c) is
   on SBUF partitions for both the Q/K (xT as rhs) and V (xT as lhsT)
   matmuls.  All matmuls run in float32r (full-rate fp32).
 - Q^T/K^T are produced as per-pair [128, 2048] tiles (head-dim on
   partitions; rows 0-63 = head 2p, 64-127 = head 2p+1), enabling row-tiled
   (K=64 x2) concurrent S^T matmuls.
 - S^T = K Q^T per (pair, k-tile, q-chunk); exp runs on ACT directly from
   PSUM with scale=1/8 folded in (no max subtraction: |scores*scale| < ~7).
 - V carries an appended ones-column so the AV matmul (out^T form, M=65)
   yields softmax denominators for free in row 64.
 - Normalization: reciprocal on DVE, broadcast via K=1 matmul with an
   all-ones lhsT, multiply on DVE; odd heads are DMA-moved to partitions
   64..127 to build the proj lhsT layout.
 - One shared PSUM slot timeline (tag "s") for qkv/V/S^T/broadcast keeps all
   phases in one pool epoch so the scheduler can overlap them freely.
"""

import numpy as np

import concourse.bacc as bacc
import concourse.bass as bass  # noqa: F401
import concourse.mybir as mybir
import concourse.tile as tile
from concourse.bass_utils import run_bass_kernel_spmd

P = 128
NQ = 2048          # sequence length
CD = 768           # model dim
NHC = 6            # heads per core
DH = 64            # head dim
SCALE = DH ** -0.5
CT = CD // P       # 6 c-tiles
KT = NQ // P       # 16 k-tiles
QC = 512           # q chunk
NQC = NQ // QC     # 4
PAIRS = NHC // 2   # 3

F32 = mybir.dt.float32
F32R = mybir.dt.float32r  # fp32 matmul mode (2 cyc/col moving operand)
BF16 = mybir.dt.bfloat16   # 1 cyc/col moving operand


def build_nc(n_reps=1, debug=False):
    nc = bacc.Bacc("TRN2", debug=False, num_devices=8)

    xT_d = nc.dram_tensor("xT", [CD, NQ], F32R, kind="ExternalInput")
    wqkvT_d = nc.dram_tensor("wqkvT", [CD, 3 * 384], F32R, kind="ExternalInput")
    bqk_d = nc.dram_tensor("b_qk", [P, 6], F32, kind="ExternalInput")
    bv_d = nc.dram_tensor("b_v", [1, 384], F32R, kind="ExternalInput")
    wpT_d = nc.dram_tensor("wpT", [384, CD], F32R, kind="ExternalInput")
    ones_d = nc.dram_tensor("ones", [P, P], F32R, kind="ExternalInput")
    out_d = nc.dram_tensor("out", [NQ, CD], F32, kind="ExternalOutput")
    if debug:
        aT_dbg = nc.dram_tensor("aT_dbg", [NQC, PAIRS, KT, P, 2, QC], F32R,
                                kind="ExternalOutput")
        qk_dbg = nc.dram_tensor("qk_dbg", [2, PAIRS, P, NQ], F32R,
                                kind="ExternalOutput")
        v_dbg = nc.dram_tensor("v_dbg", [P, KT, NHC, DH + 2], F32R,
                               kind="ExternalOutput")

    with tile.TileContext(nc) as tc:
        with (
            tc.tile_pool(name="consts", bufs=1) as consts,
            tc.tile_pool(name="big", bufs=1) as big,
            tc.tile_pool(name="attn", bufs=2) as attn_pool,
            tc.tile_pool(name="aT", bufs=4) as aT_pool,
            tc.tile_pool(name="norm", bufs=1) as norm_pool,
            tc.tile_pool(name="outst", bufs=2) as outst_pool,
            tc.tile_pool(name="ps_s", bufs=2, space="PSUM") as ps_s,
            tc.tile_pool(name="ps_av", bufs=1, space="PSUM") as ps_av,
            tc.tile_pool(name="ps_proj", bufs=1, space="PSUM") as ps_proj,
        ):
            # ---- constants (per-c-tile tiles so compute starts ASAP) ----
            xT_sb = [consts.tile([P, NQ], F32R, tag=f"xT{ct}", name=f"xT{ct}")
                     for ct in range(CT)]
            wq_sb = [consts.tile([P, 3 * 384], F32R, tag=f"wqkvT{ct}",
                                 name=f"wqkvT{ct}") for ct in range(CT)]
            for qc in range(NQC):
                for ct in range(CT):
                    nc.sync.dma_start(
                        xT_sb[ct][:, qc * QC:(qc + 1) * QC],
                        xT_d[ct * P:(ct + 1) * P, qc * QC:(qc + 1) * QC])
            for piece in (1, 0, 2):   # k cols first, then q, then v
                for ct in range(CT):
                    nc.sync.dma_start(
                        wq_sb[ct][:, piece * 384:(piece + 1) * 384],
                        wqkvT_d[ct * P:(ct + 1) * P, piece * 384:(piece + 1) * 384])
            wp_sb = []
            for t3 in range(3):
                w = consts.tile([P, CD], F32R, tag=f"wpT{t3}")
                nc.sync.dma_start(w[:, :], wpT_d[t3 * P:(t3 + 1) * P, :])
                wp_sb.append(w)
            bqk_sb = consts.tile([P, 6], F32, tag="bqk")
            nc.sync.dma_start(bqk_sb[:, :], bqk_d[:, :])
            bv_sb = consts.tile([1, 384], F32R, tag="bv")
            nc.sync.dma_start(bv_sb[:, :], bv_d[:, :])
            ones_sb = consts.tile([P, P], F32R, tag="ones")
            nc.sync.dma_start(ones_sb[:, :], ones_d[:, :])
            # -ln(64) exp bias (softmax-invariant; keeps 1/sum in fp16 range)
            expb_sb = consts.tile([P, 1], F32, tag="expb")
            nc.vector.memset(expb_sb[:, :], -4.1588830833596715)

            for _rep in range(n_reps):
                # ---- persistent activations ----
                # per-pair Q^T/K^T [128, 2048]: rows 0-63 head 2p, 64-127 head 2p+1
                q_sb = [big.tile([P, NQ], F32R, tag=f"q{p}", name=f"q{p}") for p in range(PAIRS)]
                k_sb = [big.tile([P, NQ], F32R, tag=f"k{p}", name=f"k{p}") for p in range(PAIRS)]
                # v[part=k-position, k-tile, head, 65]; col 64 = ones
                v_sb = big.tile([P, KT, NHC, DH + 2], F32R, tag="v")
                nc.sync.dma_start(
                    v_sb[:, :, :, DH],
                    ones_d[:, 0:KT * NHC].rearrange("p (a b) -> p a b", a=KT),
                )

                def qk_unit(kind, t, qc):
                    # Q^T (kind 0) / K^T (kind 1) pair-tile t, one 512-chunk
                    dest = (q_sb if kind == 0 else k_sb)[t]
                    col0 = kind * 384 + t * P
                    if True:
                        ps = ps_s.tile([P, 2, QC], F32, tag="s")
                        for ct in range(CT):
                            nc.tensor.matmul(
                                ps[:, 0, :],
                                lhsT=wq_sb[ct][:, col0:col0 + P],
                                rhs=xT_sb[ct][:, qc * QC:(qc + 1) * QC],
                                start=(ct == 0),
                                stop=(ct == CT - 1),
                            )
                        nc.vector.tensor_scalar_add(
                            out=dest[:, qc * QC:(qc + 1) * QC],
                            in0=ps[:, 0, :],
                            scalar1=bqk_sb[:, kind * 3 + t:kind * 3 + t + 1],
                        )

                def v_tile(nt):
                    ps = ps_s.tile([P, 2, QC], F32, tag="s")
                    for ct in range(CT):
                        nc.tensor.matmul(
                            ps[:, 0, 0:384],
                            lhsT=xT_sb[ct][:, nt * P:(nt + 1) * P],
                            rhs=wq_sb[ct][:, 768:1152],
                            start=(ct == 0),
                            stop=False,
                        )
                    # bias via K=1 ones-row matmul
                    nc.tensor.matmul(
                        ps[:, 0, 0:384],
                        lhsT=ones_sb[0:1, :],
                        rhs=bv_sb[0:1, :],
                        start=False,
                        stop=True,
                    )
                    nc.vector.tensor_copy(
                        out=v_sb[:, nt, :, 0:DH],
                        in_=ps[:, 0, 0:384].rearrange("p (h d) -> p h d", h=NHC),
                    )

                at_chunks = {}

                def attn_pair(qc, pr, emit_v=False, pending=None):
                    qsl = slice(qc * QC, (qc + 1) * QC)
                    if pr == 0:
                        at_chunks[qc] = attn_pool.tile([P, PAIRS, QC], F32R,
                                                       tag="attnT", name="at_chunk")
                    at_chunk = at_chunks[qc]
                    av = ps_av.tile([DH + 1, 2, QC], F32, tag="av")
                    LOOKAHEAD = 2   # S^T/exp run ahead of AV so the PE queue
                    a_ts = {}       # has work while the av slot drains

                    def st_exp(kt):
                        if emit_v:
                            v_tile(kt)
                        sp = ps_s.tile([P, 2, QC], F32, tag="s")
                        for h2 in range(2):
                            nc.tensor.matmul(
                                sp[:, h2, :],
                                lhsT=k_sb[pr][h2 * DH:(h2 + 1) * DH,
                                              kt * P:(kt + 1) * P],
                                rhs=q_sb[pr][h2 * DH:(h2 + 1) * DH, qsl],
                                start=True,
                                stop=True,
                                tile_position=(h2 * DH, 0),
                            )
                        a_t = aT_pool.tile([P, 2, QC], F32R, tag="aT")
                        # -ln(64) bias keeps 1/sum inside fp16 normal range
                        # (softmax is invariant to this common scaling)
                        nc.scalar.activation(
                            out=a_t[:, :, :],
                            in_=sp[:, :, :],
                            func=mybir.ActivationFunctionType.Exp,
                            bias=expb_sb[:, 0:1],
                            scale=float(SCALE),
                        )
                        if debug:
                            nc.sync.dma_start(aT_dbg[qc, pr, kt, :, :, :],
                                              a_t[:, :, :])
                        a_ts[kt] = a_t

                    def av_mm(kt):
                        a_t = a_ts.pop(kt)
                        for h2 in range(2):
                            nc.tensor.matmul(
                                av[:, h2, :],
                                lhsT=v_sb[:, kt, pr * 2 + h2, 0:DH + 1],
                                rhs=a_t[:, h2, :],
                                start=(kt == 0),
                                stop=(kt == KT - 1),
                            )

                    for kt in range(KT):
                        st_exp(kt)
                        if kt == 5 and pending is not None:
                            pending()   # previous pair's deferred normalization
                        if kt >= LOOKAHEAD:
                            av_mm(kt - LOOKAHEAD)
                    for kt in range(KT - LOOKAHEAD, KT):
                        av_mm(kt)
                    # evacuate av psum early (frees the slot for the next
                    # pair); the normalization itself is deferred into the
                    # next pair's kt loop so the reciprocal chain never gates
                    # the PE queue
                    avc = norm_pool.tile([DH + 1, 2, QC], F32, tag="avc")
                    nc.scalar.copy(out=avc[:, :, :], in_=av[:, :, :])

                    def norm():
                     if True:
                      if True:
                        rcf = norm_pool.tile([DH + 1, 2, QC], F32, tag="rcf")
                        rc = norm_pool.tile([DH + 1, QC], F32R, tag="rc")
                        for h2 in range(2):
                            with nc.allow_low_precision(reason="softmax denom recip"):
                                nc.vector.reciprocal(rcf[DH:DH + 1, 1, :],
                                                     avc[DH:DH + 1, h2, :])
                                nc.vector.tensor_copy(out=rc[DH:DH + 1, :],
                                                      in_=rcf[DH:DH + 1, 1, :])
                            bc_ps = ps_proj.tile([P, 2, QC], F32, tag="proj")
                            nc.tensor.matmul(
                                bc_ps[:, h2, :],
                                lhsT=ones_sb[DH:DH + 1, :],
                                rhs=rc[DH:DH + 1, :],
                                start=True,
                                stop=True,
                            )
                            if h2 == 0:
                                nc.vector.tensor_mul(
                                    out=at_chunk[0:DH, pr, :],
                                    in0=avc[0:DH, 0, :],
                                    in1=bc_ps[0:DH, h2, :],
                                )
                            else:
                                tmp = norm_pool.tile([DH, QC], F32R, tag="tmp1")
                                nc.vector.tensor_mul(
                                    out=tmp[:, :],
                                    in0=avc[0:DH, 1, :],
                                    in1=bc_ps[0:DH, h2, :],
                                )
                                nc.sync.dma_start(at_chunk[DH:P, pr, :],
                                                  tmp[:, :])
                    return norm

                def proj_chunk(qc):
                    at_chunk = at_chunks[qc]
                    for sub in range(QC // P):
                        pp = ps_proj.tile([P, 2, QC], F32, tag="proj")
                        for t3 in range(PAIRS):
                            for (bank, o0, ow) in ((0, 0, 512), (1, 512, 256)):
                                nc.tensor.matmul(
                                    pp[:, bank, 0:ow],
                                    lhsT=at_chunk[:, t3, sub * P:(sub + 1) * P],
                                    rhs=wp_sb[t3][:, o0:o0 + ow],
                                    start=(t3 == 0),
                                    stop=(t3 == PAIRS - 1),
                                )
                        ost = outst_pool.tile([P, CD], F32, tag="ost")
                        nc.vector.tensor_copy(out=ost[:, 0:512], in_=pp[:, 0, :])
                        nc.vector.tensor_copy(out=ost[:, 512:CD], in_=pp[:, 1, 0:256])
                        n0 = qc * QC + sub * P
                        nc.sync.dma_start(out_d[n0:n0 + P, :], ost[:, :])

                def qk_tile(kind, t):
                    for qc in range(NQC):
                        qk_unit(kind, t, qc)

                # software-pipelined emission: K pair0 + Q pair0 chunk0 are
                # all that chunk-0/pair-0 attention needs, so emit them first
                # (with V production interleaved per k-tile) to start ACT as
                # early as the xT DMA allows.
                qk_tile(1, 0)
                qk_unit(0, 0, 0)
                pending = attn_pair(0, 0, emit_v=True)
                for qc in range(1, NQC):
                    qk_unit(0, 0, qc)
                for pr in range(1, PAIRS):
                    qk_tile(0, pr)
                    qk_tile(1, pr)
                    pending = attn_pair(0, pr, pending=pending)
                pending()
                pending = None
                proj_chunk(0)
                for qc in range(1, NQC):
                    for pr in range(PAIRS):
                        pending = attn_pair(qc, pr, pending=pending)
                    pending()
                    pending = None
                    proj_chunk(qc)
                if debug:
                    for p in range(PAIRS):
                        nc.sync.dma_start(qk_dbg[0, p, :, :], q_sb[p][:, :])
                        nc.sync.dma_start(qk_dbg[1, p, :, :], k_sb[p][:, :])
                    nc.sync.dma_start(v_dbg[:, :, :, :], v_sb[:, :, :, :])

    nc.finalize()
    return nc


_NC = None


def _get_nc():
    global _NC
    if _NC is None:
        _NC = build_nc()
    return _NC


def _make_in_maps(inputs):
    x = np.asarray(inputs["x"], dtype=np.float32)
    w_qkv = np.asarray(inputs["w_qkv"], dtype=np.float32)
    b_qkv = np.asarray(inputs["b_qkv"], dtype=np.float32)
    w_proj = np.asarray(inputs["w_proj"], dtype=np.float32)

    in_maps = []
    for c in range(8):
        b, g = c // 2, c % 2
        sl = slice(384 * g, 384 * g + 384)
        xT = np.ascontiguousarray(x[b].T)                       # [768, 2048]
        wq = w_qkv[0:768][sl]                                    # [384, 768]
        wk = w_qkv[768:1536][sl]
        wv = w_qkv[1536:2304][sl]
        wqkvT = np.ascontiguousarray(np.concatenate([wq, wk, wv], axis=0).T)
        bq = b_qkv[0:768][sl]
        bk = b_qkv[768:1536][sl]
        bv = b_qkv[1536:2304][sl]
        b_qk = np.ascontiguousarray(
            np.concatenate([bq, bk]).reshape(6, P).T)            # [128, 6]
        wpT = np.ascontiguousarray(w_proj[:, sl].T)
        in_maps.append({
            "ones": np.ones((P, P), dtype=np.float32),
            "xT": xT,
            "wqkvT": wqkvT,
            "b_qk": b_qk,
            "b_v": np.ascontiguousarray(bv.reshape(1, 384)),
            "wpT": wpT,
        })
    return in_maps


def _run(inputs, trace=False):
    nc = _get_nc()
    in_maps = _make_in_maps(inputs)
    res = run_bass_kernel_spmd(nc, in_maps, core_ids=list(range(8)), trace=trace)
    b_proj = np.asarray(inputs["b_proj"], dtype=np.float32)
    out = np.empty((4, NQ, CD), dtype=np.float32)
    for b in range(4):
        out[b] = res.results[2 * b]["out"] + res.results[2 * b + 1]["out"] + b_proj
    return out, res


def kernel(**inputs) -> np.ndarray:
    out, _ = _run(inputs, trace=False)
    return out



# revision 5
# speedup vs baseline: 1.4641x; 1.4641x over previous
"""Multi-head attention (B=4, N=2048, D=768, H=12) on 8 trn2 NeuronCores.

Sharding: core c -> (batch b = c//2, head-half g = c%2).  Each core computes
the qkv projection for its 6 heads, attention, and a partial output
projection (over its 384 feature columns).  The host sums the two partials
per batch and adds the proj bias.  No collectives.

Device design (per core):
 - x is transposed on host to xT [768, 2048] so the contraction dim (c) is
   on SBUF partitions for both the Q/K (xT as rhs) and V (xT as lhsT)
   matmuls.  All matmul operands are bf16 (PSUM accumulation stays fp32);
   error budget is ~2e-2 so bf16's ~0.4% per-element error is safe.
 - Q^T/K^T are produced as per-pair [128, 2048] tiles (head-dim on
   partitions; rows 0-63 = head 2p, 64-127 = head 2p+1), enabling row-tiled
   (K=64 x2) concurrent S^T matmuls.
 - S^T = K Q^T per (pair, k-tile, q-chunk); exp runs on ACT directly from
   PSUM with scale=1/8 folded in (no max subtraction: |scores*scale| < ~7).
 - V carries an appended ones-column so the AV matmul (out^T form, M=65)
   yields softmax denominators for free in row 64.
 - Normalization: reciprocal_approx_fast on DVE (single op, ~18-bit),
   broadcast via K=1 matmul with an all-ones lhsT, multiply on DVE; odd
   heads are DMA-moved to partitions 64..127 to build the proj lhsT layout.
 - One shared PSUM slot timeline (tag "s") for qkv/V/S^T/broadcast keeps all
   phases in one pool epoch so the scheduler can overlap them freely.
"""

import ml_dtypes
import numpy as np

import concourse.bacc as bacc
import concourse.bass as bass  # noqa: F401
import concourse.mybir as mybir
import concourse.tile as tile
from concourse.bass_utils import run_bass_kernel_spmd

P = 128
NQ = 2048          # sequence length
CD = 768           # model dim
NHC = 6            # heads per core
DH = 64            # head dim
SCALE = DH ** -0.5
CT = CD // P       # 6 c-tiles
KT = NQ // P       # 16 k-tiles
QC = 512           # q chunk
NQC = NQ // QC     # 4
PAIRS = NHC // 2   # 3

F32 = mybir.dt.float32
F32R = mybir.dt.float32r  # fp32 matmul mode
BF16 = mybir.dt.bfloat16


def build_nc(n_reps=1, debug=False):
    nc = bacc.Bacc("TRN2", debug=False, num_devices=8)

    xT_d = nc.dram_tensor("xT", [CD, NQ], BF16, kind="ExternalInput")
    wqkvT_d = nc.dram_tensor("wqkvT", [CD, 3 * 384], BF16, kind="ExternalInput")
    bqk_d = nc.dram_tensor("b_qk", [P, 6], F32, kind="ExternalInput")
    bv_d = nc.dram_tensor("b_v", [1, 384], BF16, kind="ExternalInput")
    wpT_d = nc.dram_tensor("wpT", [384, CD], BF16, kind="ExternalInput")
    out_d = nc.dram_tensor("out", [NQ, CD], F32, kind="ExternalOutput")

    with tile.TileContext(nc) as tc:
        with (
            tc.tile_pool(name="consts", bufs=1) as consts,
            tc.tile_pool(name="big", bufs=1) as big,
            tc.tile_pool(name="attn", bufs=2) as attn_pool,
            tc.tile_pool(name="aT", bufs=4) as aT_pool,
            tc.tile_pool(name="norm", bufs=1) as norm_pool,
            tc.tile_pool(name="outst", bufs=2) as outst_pool,
            tc.tile_pool(name="ps_s", bufs=2, space="PSUM") as ps_s,
            tc.tile_pool(name="ps_av", bufs=1, space="PSUM") as ps_av,
            tc.tile_pool(name="ps_proj", bufs=1, space="PSUM") as ps_proj,
        ):
            # ---- constants (per-c-tile tiles so compute starts ASAP) ----
            xT_sb = [consts.tile([P, NQ], BF16, tag=f"xT{ct}", name=f"xT{ct}")
                     for ct in range(CT)]
            wq_sb = [consts.tile([P, 3 * 384], BF16, tag=f"wqkvT{ct}",
                                 name=f"wqkvT{ct}") for ct in range(CT)]
            for qc in range(NQC):
                for ct in range(CT):
                    nc.sync.dma_start(
                        xT_sb[ct][:, qc * QC:(qc + 1) * QC],
                        xT_d[ct * P:(ct + 1) * P, qc * QC:(qc + 1) * QC])
            for piece in (1, 0, 2):   # k cols first, then q, then v
                for ct in range(CT):
                    nc.sync.dma_start(
                        wq_sb[ct][:, piece * 384:(piece + 1) * 384],
                        wqkvT_d[ct * P:(ct + 1) * P, piece * 384:(piece + 1) * 384])
            wp_sb = []
            for t3 in range(3):
                w = consts.tile([P, CD], BF16, tag=f"wpT{t3}")
                nc.sync.dma_start(w[:, :], wpT_d[t3 * P:(t3 + 1) * P, :])
                wp_sb.append(w)
            bqk_sb = consts.tile([P, 6], F32, tag="bqk")
            nc.sync.dma_start(bqk_sb[:, :], bqk_d[:, :])
            bv_sb = consts.tile([1, 384], BF16, tag="bv")
            nc.sync.dma_start(bv_sb[:, :], bv_d[:, :])
            # ones rows for the K=1 broadcast matmuls (v-bias, reciprocal)
            onesb_sb = consts.tile([P, P], BF16, tag="onesb")
            nc.vector.memset(onesb_sb[:, :], 1.0)
            # -ln(64) exp bias (softmax-invariant; keeps 1/sum in fp16 range)
            expb_sb = consts.tile([P, 1], F32, tag="expb")
            nc.vector.memset(expb_sb[:, :], -4.1588830833596715)

            for _rep in range(n_reps):
                # ---- persistent activations ----
                # per-pair Q^T/K^T [128, 2048]: rows 0-63 head 2p, 64-127 head 2p+1
                q_sb = [big.tile([P, NQ], BF16, tag=f"q{p}", name=f"q{p}") for p in range(PAIRS)]
                k_sb = [big.tile([P, NQ], BF16, tag=f"k{p}", name=f"k{p}") for p in range(PAIRS)]
                # v[part=k-position, k-tile, head, 65]; col 64 = ones
                v_sb = big.tile([P, KT, NHC, DH + 2], BF16, tag="v")
                nc.vector.memset(v_sb[:, :, :, DH], 1.0)

                def qk_unit(kind, t, qc):
                    # Q^T (kind 0) / K^T (kind 1) pair-tile t, one 512-chunk
                    dest = (q_sb if kind == 0 else k_sb)[t]
                    col0 = kind * 384 + t * P
                    if True:
                        ps = ps_s.tile([P, 2, QC], F32, tag="s")
                        for ct in range(CT):
                            nc.tensor.matmul(
                                ps[:, 0, :],
                                lhsT=wq_sb[ct][:, col0:col0 + P],
                                rhs=xT_sb[ct][:, qc * QC:(qc + 1) * QC],
                                start=(ct == 0),
                                stop=(ct == CT - 1),
                            )
                        nc.vector.tensor_scalar_add(
                            out=dest[:, qc * QC:(qc + 1) * QC],
                            in0=ps[:, 0, :],
                            scalar1=bqk_sb[:, kind * 3 + t:kind * 3 + t + 1],
                        )

                def v_tile(nt):
                    ps = ps_s.tile([P, 2, QC], F32, tag="s")
                    for ct in range(CT):
                        nc.tensor.matmul(
                            ps[:, 0, 0:384],
                            lhsT=xT_sb[ct][:, nt * P:(nt + 1) * P],
                            rhs=wq_sb[ct][:, 768:1152],
                            start=(ct == 0),
                            stop=False,
                        )
                    # bias via K=1 ones-row matmul
                    nc.tensor.matmul(
                        ps[:, 0, 0:384],
                        lhsT=onesb_sb[0:1, :],
                        rhs=bv_sb[0:1, :],
                        start=False,
                        stop=True,
                    )
                    nc.vector.tensor_copy(
                        out=v_sb[:, nt, :, 0:DH],
                        in_=ps[:, 0, 0:384].rearrange("p (h d) -> p h d", h=NHC),
                    )

                at_chunks = {}

                def attn_pair(qc, pr, emit_v=False, pending=None):
                    qsl = slice(qc * QC, (qc + 1) * QC)
                    if pr == 0:
                        at_chunks[qc] = attn_pool.tile([P, PAIRS, QC], BF16,
                                                       tag="attnT", name="at_chunk")
                    at_chunk = at_chunks[qc]
                    av = ps_av.tile([DH + 1, 2, QC], F32, tag="av")
                    LOOKAHEAD = 2   # S^T/exp run ahead of AV so the PE queue
                    a_ts = {}       # has work while the av slot drains

                    def st_exp(kt):
                        if emit_v:
                            v_tile(kt)
                        sp = ps_s.tile([P, 2, QC], F32, tag="s")
                        for h2 in range(2):
                            nc.tensor.matmul(
                                sp[:, h2, :],
                                lhsT=k_sb[pr][h2 * DH:(h2 + 1) * DH,
                                              kt * P:(kt + 1) * P],
                                rhs=q_sb[pr][h2 * DH:(h2 + 1) * DH, qsl],
                                start=True,
                                stop=True,
                                tile_position=(h2 * DH, 0),
                            )
                        a_t = aT_pool.tile([P, 2, QC], BF16, tag="aT")
                        # -ln(64) bias keeps 1/sum inside fp16 normal range
                        # (softmax is invariant to this common scaling)
                        nc.scalar.activation(
                            out=a_t[:, :, :],
                            in_=sp[:, :, :],
                            func=mybir.ActivationFunctionType.Exp,
                            bias=expb_sb[:, 0:1],
                            scale=float(SCALE),
                        )
                        a_ts[kt] = a_t

                    def av_mm(kt):
                        a_t = a_ts.pop(kt)
                        for h2 in range(2):
                            nc.tensor.matmul(
                                av[:, h2, :],
                                lhsT=v_sb[:, kt, pr * 2 + h2, 0:DH + 1],
                                rhs=a_t[:, h2, :],
                                start=(kt == 0),
                                stop=(kt == KT - 1),
                            )

                    for kt in range(KT):
                        st_exp(kt)
                        if kt == 5 and pending is not None:
                            pending()   # previous pair's deferred normalization
                        if kt >= LOOKAHEAD:
                            av_mm(kt - LOOKAHEAD)
                    for kt in range(KT - LOOKAHEAD, KT):
                        av_mm(kt)
                    # evacuate av psum early (frees the slot for the next
                    # pair); the normalization itself is deferred into the
                    # next pair's kt loop so the reciprocal chain never gates
                    # the PE queue
                    avc = norm_pool.tile([DH + 1, 2, QC], F32, tag="avc")
                    nc.vector.tensor_copy(out=avc[:, :, :], in_=av[:, :, :])
                    # move denominators to partition 0: reciprocal_approx_fast
                    # mis-executes on partition bases > 0, and the K=1
                    # broadcast matmul wants lhsT/rhs at base 0 anyway
                    den = norm_pool.tile([1, 2, QC], F32, tag="den")
                    nc.sync.dma_start(den[0:1, :, :], avc[DH:DH + 1, :, :])

                    def norm():
                     if True:
                      if True:
                        rcf = norm_pool.tile([1, 2, QC], F32, tag="rcf")
                        rc = norm_pool.tile([1, 2, QC], BF16, tag="rc")
                        nc.vector.reciprocal_approx_fast(
                            out=rcf[0:1, :, :], in_=den[0:1, :, :])
                        nc.vector.tensor_copy(out=rc[0:1, :, :],
                                              in_=rcf[0:1, :, :])
                        for h2 in range(2):
                            bc_ps = ps_proj.tile([P, 2, QC], F32, tag="proj")
                            nc.tensor.matmul(
                                bc_ps[:, h2, :],
                                lhsT=onesb_sb[0:1, :],
                                rhs=rc[0:1, h2, :],
                                start=True,
                                stop=True,
                            )
                            if h2 == 0:
                                nc.vector.tensor_mul(
                                    out=at_chunk[0:DH, pr, :],
                                    in0=avc[0:DH, 0, :],
                                    in1=bc_ps[0:DH, h2, :],
                                )
                            else:
                                tmp = norm_pool.tile([DH, QC], BF16, tag="tmp1")
                                nc.vector.tensor_mul(
                                    out=tmp[:, :],
                                    in0=avc[0:DH, 1, :],
                                    in1=bc_ps[0:DH, h2, :],
                                )
                                nc.sync.dma_start(at_chunk[DH:P, pr, :],
                                                  tmp[:, :])
                    return norm

                def proj_chunk(qc):
                    at_chunk = at_chunks[qc]
                    for sub in range(QC // P):
                        pp = ps_proj.tile([P, 2, QC], F32, tag="proj")
                        for t3 in range(PAIRS):
                            for (bank, o0, ow) in ((0, 0, 512), (1, 512, 256)):
                                nc.tensor.matmul(
                                    pp[:, bank, 0:ow],
                                    lhsT=at_chunk[:, t3, sub * P:(sub + 1) * P],
                                    rhs=wp_sb[t3][:, o0:o0 + ow],
                                    start=(t3 == 0),
                                    stop=(t3 == PAIRS - 1),
                                )
                        ost = outst_pool.tile([P, CD], F32, tag="ost")
                        nc.vector.tensor_copy(out=ost[:, 0:512], in_=pp[:, 0, :])
                        nc.vector.tensor_copy(out=ost[:, 512:CD], in_=pp[:, 1, 0:256])
                        n0 = qc * QC + sub * P
                        nc.sync.dma_start(out_d[n0:n0 + P, :], ost[:, :])

                def qk_tile(kind, t):
                    for qc in range(NQC):
                        qk_unit(kind, t, qc)

                # software-pipelined emission: K pair0 + Q pair0 chunk0 are
                # all that chunk-0/pair-0 attention needs, so emit them first
                # (with V production interleaved per k-tile) to start ACT as
                # early as the xT DMA allows.
                qk_tile(1, 0)
                qk_unit(0, 0, 0)
                pending = attn_pair(0, 0, emit_v=True)
                for qc in range(1, NQC):
                    qk_unit(0, 0, qc)
                for pr in range(1, PAIRS):
                    qk_tile(0, pr)
                    qk_tile(1, pr)
                    pending = attn_pair(0, pr, pending=pending)
                pending()
                pending = None
                proj_chunk(0)
                for qc in range(1, NQC):
                    for pr in range(PAIRS):
                        pending = attn_pair(qc, pr, pending=pending)
                    pending()
                    pending = None
                    proj_chunk(qc)

    nc.finalize()
    return nc


_NC = None


def _get_nc():
    global _NC
    if _NC is None:
        _NC = build_nc()
    return _NC


def _make_in_maps(inputs):
    x = np.asarray(inputs["x"], dtype=np.float32)
    w_qkv = np.asarray(inputs["w_qkv"], dtype=np.float32)
    b_qkv = np.asarray(inputs["b_qkv"], dtype=np.float32)
    w_proj = np.asarray(inputs["w_proj"], dtype=np.float32)
    bf16 = ml_dtypes.bfloat16

    in_maps = []
    for c in range(8):
        b, g = c // 2, c % 2
        sl = slice(384 * g, 384 * g + 384)
        xT = np.ascontiguousarray(x[b].T).astype(bf16)               # [768, 2048]
        wq = w_qkv[0:768][sl]                                        # [384, 768]
        wk = w_qkv[768:1536][sl]
        wv = w_qkv[1536:2304][sl]
        wqkvT = np.ascontiguousarray(
            np.concatenate([wq, wk, wv], axis=0).T).astype(bf16)
        bq = b_qkv[0:768][sl]
        bk = b_qkv[768:1536][sl]
        bv = b_qkv[1536:2304][sl]
        b_qk = np.ascontiguousarray(
            np.concatenate([bq, bk]).reshape(6, P).T)                # [128, 6]
        wpT = np.ascontiguousarray(w_proj[:, sl].T).astype(bf16)
        in_maps.append({
            "xT": xT,
            "wqkvT": wqkvT,
            "b_qk": b_qk,
            "b_v": np.ascontiguousarray(bv.reshape(1, 384)).astype(bf16),
            "wpT": wpT,
        })
    return in_maps


def _run(inputs, trace=False):
    nc = _get_nc()
    in_maps = _make_in_maps(inputs)
    res = run_bass_kernel_spmd(nc, in_maps, core_ids=list(range(8)), trace=trace)
    b_proj = np.asarray(inputs["b_proj"], dtype=np.float32)
    out = np.empty((4, NQ, CD), dtype=np.float32)
    for b in range(4):
        out[b] = res.results[2 * b]["out"] + res.results[2 * b + 1]["out"] + b_proj
    return out, res


def kernel(**inputs) -> np.ndarray:
    out, _ = _run(inputs, trace=False)
    return out


# revision 6
# speedup vs baseline: 1.5514x; 1.0596x over previous
"""Multi-head attention (B=4, N=2048, D=768, H=12) on 8 trn2 NeuronCores.

Sharding: core c -> (batch b = c//2, head-half g = c%2).  Each core computes
the qkv projection for its 6 heads, attention, and a partial output
projection (over its 384 feature columns).  The host sums the two partials
per batch and adds the proj bias.  No collectives.

Device design (per core):
 - x is transposed on host to xT [768, 2048] so the contraction dim (c) is
   on SBUF partitions for both the Q/K (xT as rhs) and V (xT as lhsT)
   matmuls.  All matmul operands are bf16 (PSUM accumulation stays fp32);
   error budget is ~2e-2 so bf16's ~0.4% per-element error is safe.
 - Q^T/K^T are produced as per-pair [128, 2048] tiles (head-dim on
   partitions; rows 0-63 = head 2p, 64-127 = head 2p+1), enabling row-tiled
   (K=64 x2) concurrent S^T matmuls.
 - S^T = K Q^T per (pair, k-tile, q-chunk); exp runs on ACT directly from
   PSUM with scale=1/8 folded in (no max subtraction: |scores*scale| < ~7).
 - V carries an appended ones-column so the AV matmul (out^T form, M=65)
   yields softmax denominators for free in row 64.
 - Normalization: denominators DMA'd to partition 0 (reciprocal_approx_fast
   mis-executes on partition bases > 0), single-op reciprocal on DVE,
   broadcast via K=1 matmul with an all-ones lhsT, multiply on DVE; odd
   heads are DMA-moved to partitions 64..127 to build the proj lhsT layout.
 - Emission is software-pipelined with a per-k-tile "filler" queue: inside
   every attention kt loop (whose pace is set by ACT's exp) the PE stream is
   padded with later qk production / deferred normalization / prior-chunk
   proj sub-matmuls, so the PE never idles waiting on the exp ring and ACT
   never starves behind a PE burst.
 - Input DMAs are ordered K-weights+xT[qc0] first (interleaved per c-tile)
   so the first matmul starts within ~1 us; wp/biases ride the gpsimd
   software-DGE queue off the critical path.
"""

import ml_dtypes
import numpy as np

import concourse.bacc as bacc
import concourse.bass as bass  # noqa: F401
import concourse.mybir as mybir
import concourse.tile as tile
from concourse.bass_utils import run_bass_kernel_spmd

P = 128
NQ = 2048          # sequence length
CD = 768           # model dim
NHC = 6            # heads per core
DH = 64            # head dim
SCALE = DH ** -0.5
CT = CD // P       # 6 c-tiles
KT = NQ // P       # 16 k-tiles
QC = 512           # q chunk
NQC = NQ // QC     # 4
PAIRS = NHC // 2   # 3

F32 = mybir.dt.float32
BF16 = mybir.dt.bfloat16


def build_nc(n_reps=1, debug=False):
    nc = bacc.Bacc("TRN2", debug=False, num_devices=8)

    xT_d = nc.dram_tensor("xT", [CD, NQ], BF16, kind="ExternalInput")
    wqkvT_d = nc.dram_tensor("wqkvT", [CD, 3 * 384], BF16, kind="ExternalInput")
    bqk_d = nc.dram_tensor("b_qk", [P, 6], F32, kind="ExternalInput")
    bv_d = nc.dram_tensor("b_v", [P, 384], BF16, kind="ExternalInput")
    wpT_d = nc.dram_tensor("wpT", [384, CD], BF16, kind="ExternalInput")
    out_d = nc.dram_tensor("out", [NQ, CD], F32, kind="ExternalOutput")

    with tile.TileContext(nc) as tc:
        with (
            tc.tile_pool(name="consts", bufs=1) as consts,
            tc.tile_pool(name="big", bufs=1) as big,
            tc.tile_pool(name="attn", bufs=2) as attn_pool,
            tc.tile_pool(name="aT", bufs=4) as aT_pool,
            tc.tile_pool(name="norm", bufs=1) as norm_pool,
            tc.tile_pool(name="outst", bufs=2) as outst_pool,
            tc.tile_pool(name="ps_s", bufs=2, space="PSUM") as ps_s,
            tc.tile_pool(name="ps_av", bufs=1, space="PSUM") as ps_av,
            tc.tile_pool(name="ps_proj", bufs=1, space="PSUM") as ps_proj,
        ):
            # ---- constants ----
            xT_sb = [consts.tile([P, NQ], BF16, tag=f"xT{ct}", name=f"xT{ct}")
                     for ct in range(CT)]
            wq_sb = [consts.tile([P, 3 * 384], BF16, tag=f"wqkvT{ct}",
                                 name=f"wqkvT{ct}") for ct in range(CT)]

            def dma_wq(piece, ct):
                nc.sync.dma_start(
                    wq_sb[ct][:, piece * 384:(piece + 1) * 384],
                    wqkvT_d[ct * P:(ct + 1) * P, piece * 384:(piece + 1) * 384])

            def dma_xt(qc, ct):
                nc.sync.dma_start(
                    xT_sb[ct][:, qc * QC:(qc + 1) * QC],
                    xT_d[ct * P:(ct + 1) * P, qc * QC:(qc + 1) * QC])

            # critical-path order: K weights + xT chunk0 interleaved per
            # c-tile (first S^T-feeding matmul starts after ~2 transfers),
            # then Q weights, V weights, the rest of xT.
            for ct in range(CT):
                dma_wq(1, ct)
                dma_xt(0, ct)
            for ct in range(CT):
                dma_wq(0, ct)
            for ct in range(CT):
                dma_wq(2, ct)
            for qc in range(1, NQC):
                for ct in range(CT):
                    dma_xt(qc, ct)
            # off the critical path: gpsimd software DGE
            bqk_sb = consts.tile([P, 6], F32, tag="bqk")
            nc.gpsimd.dma_start(bqk_sb[:, :], bqk_d[:, :])
            bv_sb = consts.tile([P, 384], BF16, tag="bv")
            nc.gpsimd.dma_start(bv_sb[:, :], bv_d[:, :])
            wp_sb = []
            for t3 in range(3):
                w = consts.tile([P, CD], BF16, tag=f"wpT{t3}")
                nc.gpsimd.dma_start(w[:, :], wpT_d[t3 * P:(t3 + 1) * P, :])
                wp_sb.append(w)
            onesb_sb = consts.tile([1, P], BF16, tag="onesb")
            nc.vector.memset(onesb_sb[:, :], 1.0)
            # -ln(64) exp bias (softmax-invariant; keeps 1/sum in fp16 range)
            expb_sb = consts.tile([P, 1], F32, tag="expb")
            nc.vector.memset(expb_sb[:, :], -4.1588830833596715)

            for _rep in range(n_reps):
                # ---- persistent activations ----
                # per-pair Q^T/K^T [128, 2048]: rows 0-63 head 2p, 64-127 head 2p+1
                q_sb = [big.tile([P, NQ], BF16, tag=f"q{p}", name=f"q{p}") for p in range(PAIRS)]
                k_sb = [big.tile([P, NQ], BF16, tag=f"k{p}", name=f"k{p}") for p in range(PAIRS)]
                # v[part=k-position, k-tile, head, 65]; col 64 = ones
                v_sb = big.tile([P, KT, NHC, DH + 2], BF16, tag="v")
                nc.vector.memset(v_sb[:, :, :, DH], 1.0)

                def qk_unit(kind, t, qc):
                    # Q^T (kind 0) / K^T (kind 1) pair-tile t, one 512-chunk
                    dest = (q_sb if kind == 0 else k_sb)[t]
                    col0 = kind * 384 + t * P
                    ps = ps_s.tile([P, 2, QC], F32, tag="s")
                    for ct in range(CT):
                        nc.tensor.matmul(
                            ps[:, 0, :],
                            lhsT=wq_sb[ct][:, col0:col0 + P],
                            rhs=xT_sb[ct][:, qc * QC:(qc + 1) * QC],
                            start=(ct == 0),
                            stop=(ct == CT - 1),
                        )
                    nc.vector.tensor_scalar_add(
                        out=dest[:, qc * QC:(qc + 1) * QC],
                        in0=ps[:, 0, :],
                        scalar1=bqk_sb[:, kind * 3 + t:kind * 3 + t + 1],
                    )

                def qkf(kind, t, qc):
                    return lambda: qk_unit(kind, t, qc)

                def v_tile(nt):
                    ps = ps_s.tile([P, 2, QC], F32, tag="s")
                    for ct in range(CT):
                        nc.tensor.matmul(
                            ps[:, 0, 0:384],
                            lhsT=xT_sb[ct][:, nt * P:(nt + 1) * P],
                            rhs=wq_sb[ct][:, 768:1152],
                            start=(ct == 0),
                            stop=(ct == CT - 1),
                        )
                    # bias folded into the PSUM->SBUF evacuation
                    nc.vector.tensor_add(
                        out=v_sb[:, nt, :, 0:DH],
                        in0=ps[:, 0, 0:384].rearrange("p (h d) -> p h d", h=NHC),
                        in1=bv_sb[:, :].rearrange("p (h d) -> p h d", h=NHC),
                    )

                at_chunks = {}

                def attn_pair(qc, pr, emit_v=False, fillers=()):
                    qsl = slice(qc * QC, (qc + 1) * QC)
                    if pr == 0:
                        at_chunks[qc] = attn_pool.tile([P, PAIRS, QC], BF16,
                                                       tag="attnT", name="at_chunk")
                    at_chunk = at_chunks[qc]
                    av = ps_av.tile([DH + 1, 2, QC], F32, tag="av")
                    LOOKAHEAD = 2   # S^T/exp run ahead of AV so the PE queue
                    a_ts = {}       # has work while the av slot drains
                    fillers = list(fillers)
                    fi = 0

                    def st_exp(kt):
                        if emit_v:
                            v_tile(kt)
                        sp = ps_s.tile([P, 2, QC], F32, tag="s")
                        for h2 in range(2):
                            nc.tensor.matmul(
                                sp[:, h2, :],
                                lhsT=k_sb[pr][h2 * DH:(h2 + 1) * DH,
                                              kt * P:(kt + 1) * P],
                                rhs=q_sb[pr][h2 * DH:(h2 + 1) * DH, qsl],
                                start=True,
                                stop=True,
                                tile_position=(h2 * DH, 0),
                            )
                        a_t = aT_pool.tile([P, 2, QC], BF16, tag="aT")
                        # -ln(64) bias keeps 1/sum inside fp16 normal range
                        # (softmax is invariant to this common scaling)
                        nc.scalar.activation(
                            out=a_t[:, :, :],
                            in_=sp[:, :, :],
                            func=mybir.ActivationFunctionType.Exp,
                            bias=expb_sb[:, 0:1],
                            scale=float(SCALE),
                        )
                        a_ts[kt] = a_t

                    def av_mm(kt):
                        a_t = a_ts.pop(kt)
                        for h2 in range(2):
                            nc.tensor.matmul(
                                av[:, h2, :],
                                lhsT=v_sb[:, kt, pr * 2 + h2, 0:DH + 1],
                                rhs=a_t[:, h2, :],
                                start=(kt == 0),
                                stop=(kt == KT - 1),
                            )

                    for kt in range(KT):
                        st_exp(kt)
                        if fi < len(fillers) and kt >= 1:
                            fillers[fi]()
                            fi += 1
                        if kt >= LOOKAHEAD:
                            av_mm(kt - LOOKAHEAD)
                    for kt in range(KT - LOOKAHEAD, KT):
                        av_mm(kt)
                    while fi < len(fillers):
                        fillers[fi]()
                        fi += 1
                    # evacuate av psum early (frees the slot for the next
                    # pair); the normalization itself is deferred into the
                    # next pair's kt loop so the reciprocal chain never gates
                    # the PE queue
                    avc = norm_pool.tile([DH + 1, 2, QC], F32, tag="avc")
                    nc.vector.tensor_copy(out=avc[:, :, :], in_=av[:, :, :])
                    # move denominators to partition 0: reciprocal_approx_fast
                    # mis-executes on partition bases > 0, and the K=1
                    # broadcast matmul wants lhsT/rhs at base 0 anyway
                    den = norm_pool.tile([1, 2, QC], F32, tag="den")
                    nc.sync.dma_start(den[0:1, :, :], avc[DH:DH + 1, :, :])

                    def norm():
                        rcf = norm_pool.tile([1, 2, QC], F32, tag="rcf")
                        rc = norm_pool.tile([1, 2, QC], BF16, tag="rc")
                        nc.vector.reciprocal_approx_fast(
                            out=rcf[0:1, :, :], in_=den[0:1, :, :])
                        nc.vector.tensor_copy(out=rc[0:1, :, :],
                                              in_=rcf[0:1, :, :])
                        for h2 in range(2):
                            bc_ps = ps_proj.tile([P, 2, QC], F32, tag="proj")
                            nc.tensor.matmul(
                                bc_ps[:, h2, :],
                                lhsT=onesb_sb[0:1, :],
                                rhs=rc[0:1, h2, :],
                                start=True,
                                stop=True,
                            )
                            if h2 == 0:
                                nc.vector.tensor_mul(
                                    out=at_chunk[0:DH, pr, :],
                                    in0=avc[0:DH, 0, :],
                                    in1=bc_ps[0:DH, h2, :],
                                )
                            else:
                                tmp = norm_pool.tile([DH, QC], BF16, tag="tmp1")
                                nc.vector.tensor_mul(
                                    out=tmp[:, :],
                                    in0=avc[0:DH, 1, :],
                                    in1=bc_ps[0:DH, h2, :],
                                )
                                nc.sync.dma_start(at_chunk[DH:P, pr, :],
                                                  tmp[:, :])
                    return norm

                def proj_sub(qc, sub):
                    at_chunk = at_chunks[qc]
                    pp = ps_proj.tile([P, 2, QC], F32, tag="proj")
                    for t3 in range(PAIRS):
                        for (bank, o0, ow) in ((0, 0, 512), (1, 512, 256)):
                            nc.tensor.matmul(
                                pp[:, bank, 0:ow],
                                lhsT=at_chunk[:, t3, sub * P:(sub + 1) * P],
                                rhs=wp_sb[t3][:, o0:o0 + ow],
                                start=(t3 == 0),
                                stop=(t3 == PAIRS - 1),
                            )
                    ost = outst_pool.tile([P, CD], F32, tag="ost")
                    nc.vector.tensor_copy(out=ost[:, 0:512], in_=pp[:, 0, :])
                    nc.vector.tensor_copy(out=ost[:, 512:CD], in_=pp[:, 1, 0:256])
                    n0 = qc * QC + sub * P
                    nc.sync.dma_start(out_d[n0:n0 + P, :], ost[:, :])

                def projf(qc, sub):
                    return lambda: proj_sub(qc, sub)

                # ---- software-pipelined emission ----
                # chunk 0 warmup: k/q pair-0 chunk-0 first, then attention
                # with later qk production injected as per-kt fillers.
                qk_unit(1, 0, 0)
                qk_unit(0, 0, 0)
                n = attn_pair(0, 0, emit_v=True, fillers=[
                    qkf(1, 0, 1), qkf(1, 1, 0), qkf(0, 1, 0), qkf(1, 0, 2),
                    qkf(1, 1, 1), qkf(1, 0, 3), qkf(1, 1, 2), qkf(1, 1, 3)])
                n = attn_pair(0, 1, fillers=[
                    n, qkf(1, 2, 0), qkf(0, 2, 0), qkf(1, 2, 1), qkf(1, 2, 2),
                    qkf(1, 2, 3), qkf(0, 0, 1), qkf(0, 1, 1)])
                n = attn_pair(0, 2, fillers=[n, qkf(0, 2, 1)])
                # chunks 1..3: norms chain + prior-chunk proj + next-chunk q
                for qc in range(1, NQC):
                    nxt = qc + 1
                    f0 = [n, projf(qc - 1, 0), projf(qc - 1, 1)]
                    f1 = [projf(qc - 1, 2), projf(qc - 1, 3)]
                    f2 = []
                    if nxt < NQC:
                        f0.append(qkf(0, 0, nxt))
                        f1.append(qkf(0, 1, nxt))
                        f2.append(qkf(0, 2, nxt))
                    n = attn_pair(qc, 0, fillers=f0)
                    f1.insert(0, n)
                    n = attn_pair(qc, 1, fillers=f1)
                    f2.insert(0, n)
                    n = attn_pair(qc, 2, fillers=f2)
                n()
                for sub in range(QC // P):
                    proj_sub(NQC - 1, sub)

    nc.finalize()
    return nc


_NC = None


def _get_nc():
    global _NC
    if _NC is None:
        _NC = build_nc()
    return _NC


def _make_in_maps(inputs):
    x = np.asarray(inputs["x"], dtype=np.float32)
    w_qkv = np.asarray(inputs["w_qkv"], dtype=np.float32)
    b_qkv = np.asarray(inputs["b_qkv"], dtype=np.float32)
    w_proj = np.asarray(inputs["w_proj"], dtype=np.float32)
    bf16 = ml_dtypes.bfloat16

    in_maps = []
    for c in range(8):
        b, g = c // 2, c % 2
        sl = slice(384 * g, 384 * g + 384)
        xT = np.ascontiguousarray(x[b].T).astype(bf16)               # [768, 2048]
        wq = w_qkv[0:768][sl]                                        # [384, 768]
        wk = w_qkv[768:1536][sl]
        wv = w_qkv[1536:2304][sl]
        wqkvT = np.ascontiguousarray(
            np.concatenate([wq, wk, wv], axis=0).T).astype(bf16)
        bq = b_qkv[0:768][sl]
        bk = b_qkv[768:1536][sl]
        bv = b_qkv[1536:2304][sl]
        b_qk = np.ascontiguousarray(
            np.concatenate([bq, bk]).reshape(6, P).T)                # [128, 6]
        bv_rep = np.broadcast_to(bv.reshape(1, 384), (P, 384))
        wpT = np.ascontiguousarray(w_proj[:, sl].T).astype(bf16)
        in_maps.append({
            "xT": xT,
            "wqkvT": wqkvT,
            "b_qk": b_qk,
            "b_v": np.ascontiguousarray(bv_rep).astype(bf16),
            "wpT": wpT,
        })
    return in_maps


def _run(inputs, trace=False):
    nc = _get_nc()
    in_maps = _make_in_maps(inputs)
    res = run_bass_kernel_spmd(nc, in_maps, core_ids=list(range(8)), trace=trace)
    b_proj = np.asarray(inputs["b_proj"], dtype=np.float32)
    out = np.empty((4, NQ, CD), dtype=np.float32)
    for b in range(4):
        out[b] = res.results[2 * b]["out"] + res.results[2 * b + 1]["out"] + b_proj
    return out, res


def kernel(**inputs) -> np.ndarray:
    out, _ = _run(inputs, trace=False)
    return out
